# revision 9
# baseline (speedup 1.0000x reference)
"""AttentionSequencePoolingLayer (DIN attention) on 8 trn2 NeuronCores.

Math per (b,t): att = concat([q,k,q-k,q*k]) @ W1 + b1
  = k @ (W1b-W1c) + (q*k) @ W1d + (q@(W1a+W1c) + b1)
  = Wfold_b^T k + U_b   with  Wfold_b = (W1b-W1c) + diag(q_b) W1d  [64,80]
h1 = sigmoid(.); h2 = sigmoid(W2^T h1 + b2); s = w3.h2 (+b3); out = sum_t s_t k_t.

Device design (bf16 matmuls, data-parallel, 256 sorted slots per core):
- Mask folded into keys on host (kmask); zero keys => zero contribution, so
  masking and per-batch length truncation are free. b3 folded on host.
- Global sort by keys_length; slot s on core c processes batch rank 8s+c, so
  all cores share one static program with per-supergroup padded column
  counts C in {100, 200}.
- mm1 per slot: folded weight [65,80] (row 64 = U_b, paired with ones row in
  X) -> z1 psum, 2-bank tiles; one batched ACT sigmoid per z1 tile.
- mm2 shared W2 over 400-col chunks into psum row groups {0,64}; one ACT per
  z2 bank; mm3 block-diag w3 -> scores at psum row groups {0,32,64,96}.
- PE transpose of score strips (with junk rows) -> token-major scores; DVE
  copies; per-slot pooling matmuls into psum (row 32*(m%4), col 64*(m//4));
  strided-partition DMA of rows {0,32,64,96} to HBM.
"""
import numpy as np
import ml_dtypes

import concourse.bacc as bacc
import concourse.bass as bass
import concourse.mybir as mybir
import concourse.tile as tile
from concourse.bass_utils import run_bass_kernel_spmd

B, T, E = 2048, 200, 64
H1, H2 = 80, 40
NCORES = 8
NSLOT = B // NCORES          # 256 slots per core
SGS = 16                     # slots per supergroup
NSG = NSLOT // SGS           # 16 supergroups

bf16 = mybir.dt.bfloat16
f32 = mybir.dt.float32
nbf = ml_dtypes.bfloat16
SIG = mybir.ActivationFunctionType.Sigmoid

_cache = {}


def _to_bf16(x: np.ndarray) -> np.ndarray:
    """Fast float32 -> bfloat16 (round-to-nearest-even) via integer math."""
    x = np.ascontiguousarray(x, dtype=np.float32)
    u = x.view(np.uint32)
    r = ((u + 0x7FFF + ((u >> 16) & 1)) >> 16).astype(np.uint16)
    return r.view(nbf).reshape(x.shape)


def _build(cprof):
    """cprof: tuple of NSG supergroup column counts (100 or 200)."""
    nc = bacc.Bacc(None, target_bir_lowering=False)

    xc = sum(SGS * c for c in cprof)
    kc = sum(SGS * (c // 100) * 64 for c in cprof)

    x_d = nc.dram_tensor("xf", [65, xc], bf16, kind="ExternalInput")
    k_d = nc.dram_tensor("kt", [100, kc], bf16, kind="ExternalInput")
    w_d = nc.dram_tensor("wf", [65, NSLOT * H1], bf16, kind="ExternalInput")
    w2_d = nc.dram_tensor("w2", [128, 64], bf16, kind="ExternalInput")
    w3_d = nc.dram_tensor("w3b", [128, 32], bf16, kind="ExternalInput")
    b2_d = nc.dram_tensor("b2r", [128, 1], f32, kind="ExternalInput")
    id_d = nc.dram_tensor("idn", [98, 98], bf16, kind="ExternalInput")
    out_d = nc.dram_tensor("out", [NSLOT // 32, 4, 512], f32, kind="ExternalOutput")

    with tile.TileContext(nc) as tc:
        with (
            tc.tile_pool(name="const", bufs=1) as const,
            tc.tile_pool(name="xin", bufs=4) as xin,
            tc.tile_pool(name="kin", bufs=4) as kin,
            tc.tile_pool(name="win", bufs=4) as win,
            tc.tile_pool(name="h1p", bufs=2) as h1p,
            tc.tile_pool(name="h2p", bufs=2) as h2p,
            tc.tile_pool(name="ssp", bufs=2) as ssp,
            tc.tile_pool(name="stp", bufs=6) as stp,
            tc.tile_pool(name="stg", bufs=2) as stg,
            tc.tile_pool(name="z1p", bufs=2, space=bass.MemorySpace.PSUM) as z1p,
            tc.tile_pool(name="z2p", bufs=1, space=bass.MemorySpace.PSUM) as z2p,
            tc.tile_pool(name="spp", bufs=1, space=bass.MemorySpace.PSUM) as spp,
            tc.tile_pool(name="plp", bufs=2, space=bass.MemorySpace.PSUM) as plp,
        ):
            w2_s = const.tile([128, 64], bf16)
            w3_s = const.tile([128, 32], bf16)
            b2_s = const.tile([128, 1], f32)
            id_s = const.tile([98, 98], bf16)
            nc.gpsimd.dma_start(w2_s[:], w2_d[:])
            nc.gpsimd.dma_start(w3_s[:], w3_d[:])
            nc.gpsimd.dma_start(b2_s[:], b2_d[:])
            nc.gpsimd.dma_start(id_s[:], id_d[:])

            XW = 3200   # max SGS*C columns
            for _ in range(4):
                t0 = xin.tile([128, XW], bf16, tag="xt")
                nc.vector.memset(t0[64:128, :], 0.0)
                t1 = win.tile([128, SGS * H1], bf16, tag="wt")
                nc.vector.memset(t1[64:128, :], 0.0)
            for _ in range(2):
                t2 = h1p.tile([128, XW], bf16, tag="h1t")
                nc.vector.memset(t2[64:128, :], 0.0)

            xoff = 0
            koff = 0
            poolt = None
            for g in range(NSG):
                C = cprof[g]
                nblk = C // 100
                nch = 400 // C            # slots per 400-col mm2 chunk
                chunks = SGS // nch       # 4 (C=100) or 8 (C=200)
                zb = chunks // 2          # z2 banks / mm3 count: 2 or 4
                srows = 32 * (zb - 1) + 2

                if g % 2 == 0:
                    poolt = plp.tile([97, 512], f32, tag="poolt")

                xt = xin.tile([128, XW], bf16, tag="xt")
                kt = kin.tile([100, SGS * nblk * 64], bf16, tag="ktt")
                wt = win.tile([128, SGS * H1], bf16, tag="wt")
                if g == 0:
                    q4 = SGS * C // 4
                    for qq in range(4):
                        nc.sync.dma_start(
                            xt[0:65, qq * q4 : (qq + 1) * q4],
                            x_d[:, xoff + qq * q4 : xoff + (qq + 1) * q4],
                        )
                else:
                    nc.sync.dma_start(
                        xt[0:65, 0 : SGS * C], x_d[:, xoff : xoff + SGS * C]
                    )
                nc.gpsimd.dma_start(kt[:], k_d[:, koff : koff + SGS * nblk * 64])
                nc.gpsimd.dma_start(wt[0:65, :], w_d[:, g * SGS * H1 : (g + 1) * SGS * H1])

                # ---- layer 1: per-slot folded matmul + batched sigmoid ----
                h1t = h1p.tile([128, XW], bf16, tag="h1t")
                spb = 8 // nblk           # slots per 2-bank z1 tile
                cstr = 128 if C == 100 else 256
                for zt in range(SGS // spb):
                    z1 = z1p.tile([H1, 2, 512], f32, tag="z1")
                    for i in range(spb):
                        s = zt * spb + i
                        bb = i // (spb // 2)
                        j = i % (spb // 2)
                        nc.tensor.matmul(
                            z1[:, bb, j * cstr : j * cstr + C],
                            wt[:, s * H1 : (s + 1) * H1],
                            xt[:, s * C : (s + 1) * C],
                            start=True,
                            stop=True,
                        )
                    zv = z1[:].rearrange("p b (j c) -> p b j c", c=cstr)
                    hv = h1t[0:H1, zt * spb * C : (zt + 1) * spb * C].rearrange(
                        "p (b j c) -> p b j c", b=2, c=C
                    )
                    nc.scalar.activation(hv, zv[:, :, :, 0:C], SIG)

                # ---- layer 2 + scores ----
                h2t = h2p.tile([128, zb * 400], bf16, tag="h2t")
                spt = spp.tile([128, 512], f32, tag="spt")
                for v in range(zb):
                    z2 = z2p.tile([128, 400], f32, tag="z2")
                    nc.tensor.matmul(
                        z2[0:64, 0:400],
                        w2_s[:],
                        h1t[:, (2 * v) * 400 : (2 * v) * 400 + 400],
                        start=True,
                        stop=True,
                    )
                    nc.tensor.matmul(
                        z2[64:128, 0:400],
                        w2_s[:],
                        h1t[:, (2 * v + 1) * 400 : (2 * v + 1) * 400 + 400],
                        start=True,
                        stop=True,
                        tile_position=(0, 64),
                    )
                    nc.scalar.activation(
                        h2t[:, v * 400 : (v + 1) * 400], z2[:], SIG,
                        bias=b2_s[:, 0:1],
                    )
                    nc.tensor.matmul(
                        spt[32 * v : 32 * v + 32, 0:400],
                        w3_s[:],
                        h2t[:, v * 400 : (v + 1) * 400],
                        start=True,
                        stop=True,
                        tile_position=(0, 32 * v),
                    )

                # ---- transpose scores to token-major + pooling ----
                sst = ssp.tile([98, 400], bf16, tag="sst")
                nc.vector.tensor_copy(sst[0:srows, :], spt[0:srows, 0:400])
                stts = []
                for w in range(4):
                    stt = stp.tile([100, 98], bf16, tag="stt")
                    stts.append(stt)
                    bfv = spt[0:100, 400:449].bitcast(bf16)
                    nc.tensor.transpose(
                        bfv[:, 0:srows],
                        sst[0:srows, 100 * w : 100 * w + 100],
                        id_s[0:srows, 0:srows],
                    )
                    nc.vector.tensor_copy(stt[:, 0:srows], bfv[:, 0:srows])
                    # pooling: both blocks of a slot must be issued
                    # back-to-back (an interleaved start=True to the same
                    # psum rows clears the open group's has_written).
                    if C == 100:
                        for u in range(chunks):
                            m = 4 * u + w
                            M32 = (g % 2) * 16 + m
                            r, cc = M32 % 4, M32 // 4
                            col = 32 * (u // 2) + (u % 2)
                            nc.tensor.matmul(
                                poolt[32 * r : 32 * r + 1, 64 * cc : 64 * cc + 64],
                                stt[:, col : col + 1],
                                kt[:, (m * nblk) * 64 : (m * nblk + 1) * 64],
                                start=True,
                                stop=True,
                                tile_position=(0, 32 * r),
                            )
                    elif w % 2 == 1:
                        for u in range(chunks):
                            m = 2 * u + w // 2
                            M32 = (g % 2) * 16 + m
                            r, cc = M32 % 4, M32 // 4
                            col = 32 * (u // 2) + (u % 2)
                            for blk in (0, 1):
                                nc.tensor.matmul(
                                    poolt[32 * r : 32 * r + 1, 64 * cc : 64 * cc + 64],
                                    stts[w - 1 + blk][:, col : col + 1],
                                    kt[:, (m * nblk + blk) * 64 : (m * nblk + blk + 1) * 64],
                                    start=(blk == 0),
                                    stop=(blk == 1),
                                    tile_position=(0, 32 * r),
                                )

                xoff += SGS * C
                koff += SGS * nblk * 64

                # ---- drain pool psum every 2 supergroups ----
                if g % 2 == 1:
                    stage = stg.tile([97, 512], f32, tag="stage")
                    nc.vector.tensor_copy(stage[:], poolt[0:97, :])
                    nc.sync.dma_start(out_d[g // 2, :, :], stage[0:97:32, :])

    nc.compile()
    return nc


def kernel(query, keys, keys_length, W1, b1, W2, b2, W3, b3):
    query = np.asarray(query, np.float32)
    keys = np.asarray(keys, np.float32)
    keys_length = np.asarray(keys_length, np.int32)
    W1 = np.asarray(W1, np.float32)
    b1 = np.asarray(b1, np.float32)
    W2 = np.asarray(W2, np.float32)
    b2 = np.asarray(b2, np.float32)
    W3 = np.asarray(W3, np.float32)
    b3 = np.asarray(b3, np.float32)

    lens = keys_length[:, 0]
    order = np.argsort(lens, kind="stable")
    slot_max = lens[order[7::8]]                     # max len per slot
    C_s = np.where(slot_max <= 100, 100, 200)
    n1 = int((C_s == 100).sum())
    n1f = (n1 // SGS) * SGS
    cprof = tuple(100 if (g + 1) * SGS <= n1f else 200 for g in range(NSG))

    if cprof not in _cache:
        _cache[cprof] = _build(cprof)
    nc = _cache[cprof]

    # host-side data prep
    A = W1[0:E] + W1[2 * E : 3 * E]
    Bw = W1[E : 2 * E] - W1[2 * E : 3 * E]
    Cm = W1[3 * E : 4 * E]
    q2 = query[:, 0, :]                              # [B, E]
    mask = (np.arange(T)[None, :] < lens[:, None]).astype(np.float32)
    kmask = keys * mask[:, :, None]                  # [B, T, E]
    U = q2 @ A + b1                                  # [B, H1]
    Wf = Bw[None] + q2[:, :, None] * Cm[None]        # [B, E, H1]

    w2pad = np.zeros((128, 64), np.float32)
    w2pad[0:H1, 0:H2] = W2
    w2b = _to_bf16(w2pad)
    w3blk = np.zeros((128, 32), np.float32)
    w3blk[0:H2, 0] = W3[:, 0]
    w3blk[64 : 64 + H2, 1] = W3[:, 0]
    w3b = _to_bf16(w3blk)
    b2r = np.zeros((128, 1), np.float32)
    b2r[0:H2, 0] = b2
    b2r[64 : 64 + H2, 0] = b2
    idn = _to_bf16(np.eye(98, dtype=np.float32))

    in_maps = []
    core_ids_all = []
    for c in range(NCORES):
        ids = order[np.arange(NSLOT) * 8 + c]
        core_ids_all.append(ids)
        km = kmask[ids]                              # [256, T, E]
        wf = Wf[ids]                                 # [256, E, H1]
        uu = U[ids]                                  # [256, H1]

        xparts = []
        kparts = []
        for g in range(NSG):
            C = int(cprof[g])
            nblk = C // 100
            sl = slice(g * SGS, (g + 1) * SGS)
            kg = km[sl, 0:C, :]                      # [16, C, 64]
            xg = np.empty((65, SGS * C), np.float32)
            xg[0:64] = kg.transpose(2, 0, 1).reshape(64, SGS * C)
            xg[64] = 1.0
            xparts.append(xg)
            kparts.append(
                kg.reshape(SGS, nblk, 100, 64)
                .transpose(2, 0, 1, 3)
                .reshape(100, SGS * nblk * 64)
            )
        xf = _to_bf16(np.concatenate(xparts, axis=1))
        ktk = _to_bf16(np.concatenate(kparts, axis=1))

        wfull = np.empty((65, NSLOT * H1), np.float32)
        wfull[0:64] = wf.transpose(1, 0, 2).reshape(64, NSLOT * H1)
        wfull[64] = uu.reshape(NSLOT * H1)
        wfb = _to_bf16(wfull)

        in_maps.append({
            "xf": xf, "kt": ktk, "wf": wfb, "w2": w2b, "w3b": w3b,
            "b2r": b2r, "idn": idn,
        })

    res = run_bass_kernel_spmd(nc, in_maps, list(range(NCORES)))

    out_full = np.empty((B, E), np.float32)
    for c in range(NCORES):
        o = np.asarray(res.results[c]["out"])        # [8, 4, 512]
        rows = o.reshape(8, 4, 8, 64).transpose(0, 2, 1, 3).reshape(NSLOT, 64)
        out_full[core_ids_all[c]] = rows
    b3f = float(b3.reshape(-1)[0])
    if b3f != 0.0:
        out_full += b3f * kmask.sum(axis=1)
    return out_full.reshape(B, 1, E).astype(np.float32)


# revision 10
# speedup vs baseline: 1.0461x; 1.0461x over previous
"""AttentionSequencePoolingLayer (DIN attention) on 8 trn2 NeuronCores.

Math per (b,t): att = concat([q,k,q-k,q*k]) @ W1 + b1
  = k @ (W1b-W1c) + (q*k) @ W1d + (q@(W1a+W1c) + b1)
  = Wfold_b^T k + U_b   with  Wfold_b = (W1b-W1c) + diag(q_b) W1d  [64,80]
h1 = sigmoid(.); h2 = sigmoid(W2^T h1 + b2); s = w3.h2 (+b3); out = sum_t s_t k_t.

Device design (bf16 matmuls, data-parallel, 256 sorted slots per core):
- Mask folded into keys on host (kmask); zero keys => zero contribution, so
  masking and per-batch length truncation are free. b3 folded on host.
- Global sort by keys_length; slot s on core c processes batch rank 8s+c, so
  all cores share one static program with per-supergroup padded column
  counts C in {100, 200}.
- mm1 per slot: folded weight [65,80] (row 64 = U_b, paired with ones row in
  X) -> z1 psum, 2-bank tiles; one batched ACT sigmoid per z1 tile.
- mm2 shared W2 over 400-col chunks into psum row groups {0,64}; one ACT per
  z2 bank; mm3 block-diag w3 -> scores at psum row groups {0,32,64,96}.
- PE transpose of score strips (with junk rows) -> token-major scores; DVE
  copies; per-slot pooling matmuls into psum (row 32*(m%4), col 64*(m//4));
  strided-partition DMA of rows {0,32,64,96} to HBM.
"""
import numpy as np
import ml_dtypes

import concourse.bacc as bacc
import concourse.bass as bass
import concourse.mybir as mybir
import concourse.tile as tile
from concourse.bass_utils import run_bass_kernel_spmd

B, T, E = 2048, 200, 64
H1, H2 = 80, 40
NCORES = 8
NSLOT = B // NCORES          # 256 slots per core
SGS = 16                     # slots per supergroup
NSG = NSLOT // SGS           # 16 supergroups

bf16 = mybir.dt.bfloat16
f32 = mybir.dt.float32
nbf = ml_dtypes.bfloat16
SIG = mybir.ActivationFunctionType.Sigmoid

_cache = {}


def _to_bf16(x: np.ndarray) -> np.ndarray:
    """Fast float32 -> bfloat16 (round-to-nearest-even) via integer math."""
    x = np.ascontiguousarray(x, dtype=np.float32)
    u = x.view(np.uint32)
    r = ((u + 0x7FFF + ((u >> 16) & 1)) >> 16).astype(np.uint16)
    return r.view(nbf).reshape(x.shape)


def _build(cprof):
    """cprof: tuple of NSG supergroup column counts (100 or 200)."""
    nc = bacc.Bacc(None, target_bir_lowering=False)

    xc = sum(SGS * c for c in cprof)
    kc = sum(SGS * (c // 100) * 64 for c in cprof)

    x_d = nc.dram_tensor("xf", [65, xc], bf16, kind="ExternalInput")
    k_d = nc.dram_tensor("kt", [100, kc], bf16, kind="ExternalInput")
    w_d = nc.dram_tensor("wf", [65, NSLOT * H1], bf16, kind="ExternalInput")
    w2_d = nc.dram_tensor("w2", [128, 64], bf16, kind="ExternalInput")
    w3_d = nc.dram_tensor("w3b", [128, 32], bf16, kind="ExternalInput")
    b2_d = nc.dram_tensor("b2r", [128, 1], f32, kind="ExternalInput")
    id_d = nc.dram_tensor("idn", [98, 98], bf16, kind="ExternalInput")
    out_d = nc.dram_tensor("out", [NSLOT // 32, 4, 512], f32, kind="ExternalOutput")

    with tile.TileContext(nc) as tc:
        with (
            tc.tile_pool(name="const", bufs=1) as const,
            tc.tile_pool(name="xin", bufs=4) as xin,
            tc.tile_pool(name="kin", bufs=4) as kin,
            tc.tile_pool(name="win", bufs=4) as win,
            tc.tile_pool(name="h1p", bufs=2) as h1p,
            tc.tile_pool(name="h2p", bufs=2) as h2p,
            tc.tile_pool(name="ssp", bufs=2) as ssp,
            tc.tile_pool(name="stp", bufs=6) as stp,
            tc.tile_pool(name="stg", bufs=2) as stg,
            tc.tile_pool(name="z1p", bufs=2, space=bass.MemorySpace.PSUM) as z1p,
            tc.tile_pool(name="z2p", bufs=1, space=bass.MemorySpace.PSUM) as z2p,
            tc.tile_pool(name="spp", bufs=1, space=bass.MemorySpace.PSUM) as spp,
            tc.tile_pool(name="plp", bufs=2, space=bass.MemorySpace.PSUM) as plp,
        ):
            w2_s = const.tile([128, 64], bf16)
            w3_s = const.tile([128, 32], bf16)
            b2_s = const.tile([128, 1], f32)
            id_s = const.tile([98, 98], bf16)
            nc.gpsimd.dma_start(w2_s[:], w2_d[:])
            nc.gpsimd.dma_start(w3_s[:], w3_d[:])
            nc.gpsimd.dma_start(b2_s[:], b2_d[:])
            nc.gpsimd.dma_start(id_s[:], id_d[:])

            XW = 3200   # max SGS*C columns
            for _ in range(4):
                t0 = xin.tile([128, XW], bf16, tag="xt")
                nc.vector.memset(t0[64:128, :], 0.0)
                t1 = win.tile([128, SGS * H1], bf16, tag="wt")
                nc.vector.memset(t1[64:128, :], 0.0)
            for _ in range(2):
                t2 = h1p.tile([128, XW], bf16, tag="h1t")
                nc.vector.memset(t2[64:128, :], 0.0)

            xoff = 0
            koff = 0
            poolt = None
            for g in range(NSG):
                C = cprof[g]
                nblk = C // 100
                nch = 400 // C            # slots per 400-col mm2 chunk
                chunks = SGS // nch       # 4 (C=100) or 8 (C=200)
                zb = chunks // 2          # z2 banks / mm3 count: 2 or 4
                srows = 32 * (zb - 1) + 2

                if g % 2 == 0:
                    poolt = plp.tile([97, 512], f32, tag="poolt")

                xt = xin.tile([128, XW], bf16, tag="xt")
                kt = kin.tile([100, SGS * nblk * 64], bf16, tag="ktt")
                wt = win.tile([128, SGS * H1], bf16, tag="wt")
                if g == 0:
                    nc.sync.dma_start(
                        wt[0:65, :], w_d[:, g * SGS * H1 : (g + 1) * SGS * H1]
                    )
                    q4 = SGS * C // 4
                    for qq in range(4):
                        nc.sync.dma_start(
                            xt[0:65, qq * q4 : (qq + 1) * q4],
                            x_d[:, xoff + qq * q4 : xoff + (qq + 1) * q4],
                        )
                else:
                    nc.sync.dma_start(
                        xt[0:65, 0 : SGS * C], x_d[:, xoff : xoff + SGS * C]
                    )
                    nc.gpsimd.dma_start(
                        wt[0:65, :], w_d[:, g * SGS * H1 : (g + 1) * SGS * H1]
                    )
                nc.gpsimd.dma_start(kt[:], k_d[:, koff : koff + SGS * nblk * 64])

                # ---- layer 1: per-slot folded matmul + batched sigmoid ----
                h1t = h1p.tile([128, XW], bf16, tag="h1t")
                spb = 8 // nblk           # slots per 2-bank z1 tile
                cstr = 128 if C == 100 else 256
                for zt in range(SGS // spb):
                    z1 = z1p.tile([H1, 2, 512], f32, tag="z1")
                    for i in range(spb):
                        s = zt * spb + i
                        bb = i // (spb // 2)
                        j = i % (spb // 2)
                        nc.tensor.matmul(
                            z1[:, bb, j * cstr : j * cstr + C],
                            wt[:, s * H1 : (s + 1) * H1],
                            xt[:, s * C : (s + 1) * C],
                            start=True,
                            stop=True,
                        )
                    zv = z1[:].rearrange("p b (j c) -> p b j c", c=cstr)
                    hv = h1t[0:H1, zt * spb * C : (zt + 1) * spb * C].rearrange(
                        "p (b j c) -> p b j c", b=2, c=C
                    )
                    nc.scalar.activation(hv, zv[:, :, :, 0:C], SIG)

                # ---- layer 2 + scores ----
                h2t = h2p.tile([128, zb * 400], bf16, tag="h2t")
                spt = spp.tile([128, 512], f32, tag="spt")
                for v in range(zb):
                    z2 = z2p.tile([128, 400], f32, tag="z2")
                    nc.tensor.matmul(
                        z2[0:64, 0:400],
                        w2_s[:],
                        h1t[:, (2 * v) * 400 : (2 * v) * 400 + 400],
                        start=True,
                        stop=True,
                    )
                    nc.tensor.matmul(
                        z2[64:128, 0:400],
                        w2_s[:],
                        h1t[:, (2 * v + 1) * 400 : (2 * v + 1) * 400 + 400],
                        start=True,
                        stop=True,
                        tile_position=(0, 64),
                    )
                    nc.scalar.activation(
                        h2t[:, v * 400 : (v + 1) * 400], z2[:], SIG,
                        bias=b2_s[:, 0:1],
                    )
                    nc.tensor.matmul(
                        spt[32 * v : 32 * v + 32, 0:400],
                        w3_s[:],
                        h2t[:, v * 400 : (v + 1) * 400],
                        start=True,
                        stop=True,
                        tile_position=(0, 32 * v),
                    )

                # ---- transpose scores to token-major + pooling ----
                sst = ssp.tile([98, 400], bf16, tag="sst")
                nc.vector.tensor_copy(sst[0:srows, :], spt[0:srows, 0:400])
                stts = []
                for w in range(4):
                    stt = stp.tile([100, 98], bf16, tag="stt")
                    stts.append(stt)
                    bfv = spt[0:100, 400:449].bitcast(bf16)
                    nc.tensor.transpose(
                        bfv[:, 0:srows],
                        sst[0:srows, 100 * w : 100 * w + 100],
                        id_s[0:srows, 0:srows],
                    )
                    nc.vector.tensor_copy(stt[:, 0:srows], bfv[:, 0:srows])
                    # pooling: both blocks of a slot must be issued
                    # back-to-back (an interleaved start=True to the same
                    # psum rows clears the open group's has_written).
                    if C == 100:
                        for u in range(chunks):
                            m = 4 * u + w
                            M32 = (g % 2) * 16 + m
                            r, cc = M32 % 4, M32 // 4
                            col = 32 * (u // 2) + (u % 2)
                            nc.tensor.matmul(
                                poolt[32 * r : 32 * r + 1, 64 * cc : 64 * cc + 64],
                                stt[:, col : col + 1],
                                kt[:, (m * nblk) * 64 : (m * nblk + 1) * 64],
                                start=True,
                                stop=True,
                                tile_position=(0, 32 * r),
                            )
                    elif w % 2 == 1:
                        for u in range(chunks):
                            m = 2 * u + w // 2
                            M32 = (g % 2) * 16 + m
                            r, cc = M32 % 4, M32 // 4
                            col = 32 * (u // 2) + (u % 2)
                            for blk in (0, 1):
                                nc.tensor.matmul(
                                    poolt[32 * r : 32 * r + 1, 64 * cc : 64 * cc + 64],
                                    stts[w - 1 + blk][:, col : col + 1],
                                    kt[:, (m * nblk + blk) * 64 : (m * nblk + blk + 1) * 64],
                                    start=(blk == 0),
                                    stop=(blk == 1),
                                    tile_position=(0, 32 * r),
                                )

                xoff += SGS * C
                koff += SGS * nblk * 64

                # ---- drain pool psum every 2 supergroups ----
                if g % 2 == 1:
                    stage = stg.tile([97, 512], f32, tag="stage")
                    nc.vector.tensor_copy(stage[:], poolt[0:97, :])
                    nc.sync.dma_start(out_d[g // 2, :, :], stage[0:97:32, :])

    nc.compile()
    return nc


def kernel(query, keys, keys_length, W1, b1, W2, b2, W3, b3):
    query = np.asarray(query, np.float32)
    keys = np.asarray(keys, np.float32)
    keys_length = np.asarray(keys_length, np.int32)
    W1 = np.asarray(W1, np.float32)
    b1 = np.asarray(b1, np.float32)
    W2 = np.asarray(W2, np.float32)
    b2 = np.asarray(b2, np.float32)
    W3 = np.asarray(W3, np.float32)
    b3 = np.asarray(b3, np.float32)

    lens = keys_length[:, 0]
    order = np.argsort(lens, kind="stable")[::-1]    # descending length
    slot_max = lens[order[0::8]]                     # max len per slot
    C_s = np.where(slot_max <= 100, 100, 200)
    n1 = int((C_s == 100).sum())                     # trailing short slots
    n1f = (n1 // SGS) * SGS
    cprof = tuple(100 if g * SGS >= NSLOT - n1f else 200 for g in range(NSG))

    if cprof not in _cache:
        _cache[cprof] = _build(cprof)
    nc = _cache[cprof]

    # host-side data prep
    A = W1[0:E] + W1[2 * E : 3 * E]
    Bw = W1[E : 2 * E] - W1[2 * E : 3 * E]
    Cm = W1[3 * E : 4 * E]
    q2 = query[:, 0, :]                              # [B, E]
    mask = (np.arange(T)[None, :] < lens[:, None]).astype(np.float32)
    kmask = keys * mask[:, :, None]                  # [B, T, E]
    U = q2 @ A + b1                                  # [B, H1]
    Wf = Bw[None] + q2[:, :, None] * Cm[None]        # [B, E, H1]

    w2pad = np.zeros((128, 64), np.float32)
    w2pad[0:H1, 0:H2] = W2
    w2b = _to_bf16(w2pad)
    w3blk = np.zeros((128, 32), np.float32)
    w3blk[0:H2, 0] = W3[:, 0]
    w3blk[64 : 64 + H2, 1] = W3[:, 0]
    w3b = _to_bf16(w3blk)
    b2r = np.zeros((128, 1), np.float32)
    b2r[0:H2, 0] = b2
    b2r[64 : 64 + H2, 0] = b2
    idn = _to_bf16(np.eye(98, dtype=np.float32))

    in_maps = []
    core_ids_all = []
    for c in range(NCORES):
        ids = order[np.arange(NSLOT) * 8 + c]
        core_ids_all.append(ids)
        km = kmask[ids]                              # [256, T, E]
        wf = Wf[ids]                                 # [256, E, H1]
        uu = U[ids]                                  # [256, H1]

        xparts = []
        kparts = []
        for g in range(NSG):
            C = int(cprof[g])
            nblk = C // 100
            sl = slice(g * SGS, (g + 1) * SGS)
            kg = km[sl, 0:C, :]                      # [16, C, 64]
            xg = np.empty((65, SGS * C), np.float32)
            xg[0:64] = kg.transpose(2, 0, 1).reshape(64, SGS * C)
            xg[64] = 1.0
            xparts.append(xg)
            kparts.append(
                kg.reshape(SGS, nblk, 100, 64)
                .transpose(2, 0, 1, 3)
                .reshape(100, SGS * nblk * 64)
            )
        xf = _to_bf16(np.concatenate(xparts, axis=1))
        ktk = _to_bf16(np.concatenate(kparts, axis=1))

        wfull = np.empty((65, NSLOT * H1), np.float32)
        wfull[0:64] = wf.transpose(1, 0, 2).reshape(64, NSLOT * H1)
        wfull[64] = uu.reshape(NSLOT * H1)
        wfb = _to_bf16(wfull)

        in_maps.append({
            "xf": xf, "kt": ktk, "wf": wfb, "w2": w2b, "w3b": w3b,
            "b2r": b2r, "idn": idn,
        })

    res = run_bass_kernel_spmd(nc, in_maps, list(range(NCORES)))

    out_full = np.empty((B, E), np.float32)
    for c in range(NCORES):
        o = np.asarray(res.results[c]["out"])        # [8, 4, 512]
        rows = o.reshape(8, 4, 8, 64).transpose(0, 2, 1, 3).reshape(NSLOT, 64)
        out_full[core_ids_all[c]] = rows
    b3f = float(b3.reshape(-1)[0])
    if b3f != 0.0:
        out_full += b3f * kmask.sum(axis=1)
    return out_full.reshape(B, 1, E).astype(np.float32)


# revision 11
# speedup vs baseline: 1.0463x; 1.0002x over previous
"""AttentionSequencePoolingLayer (DIN attention) on 8 trn2 NeuronCores.

Math per (b,t): att = concat([q,k,q-k,q*k]) @ W1 + b1
  = k @ (W1b-W1c) + (q*k) @ W1d + (q@(W1a+W1c) + b1)
  = Wfold_b^T k + U_b   with  Wfold_b = (W1b-W1c) + diag(q_b) W1d  [64,80]
h1 = sigmoid(.); h2 = sigmoid(W2^T h1 + b2); s = w3.h2 (+b3); out = sum_t s_t k_t.

Device design (bf16 matmuls, data-parallel, 256 sorted slots per core):
- Mask folded into keys on host (kmask); zero keys => zero contribution, so
  masking and per-batch length truncation are free. b3 folded on host.
- Global sort by keys_length; slot s on core c processes batch rank 8s+c, so
  all cores share one static program with per-supergroup padded column
  counts C in {100, 200}.
- mm1 per slot: folded weight [65,80] (row 64 = U_b, paired with ones row in
  X) -> z1 psum, 2-bank tiles; one batched ACT sigmoid per z1 tile.
- mm2 shared W2 over 400-col chunks into psum row groups {0,64}; one ACT per
  z2 bank; mm3 block-diag w3 -> scores at psum row groups {0,32,64,96}.
- PE transpose of score strips (with junk rows) -> token-major scores; DVE
  copies; per-slot pooling matmuls into psum (row 32*(m%4), col 64*(m//4));
  strided-partition DMA of rows {0,32,64,96} to HBM.
"""
import numpy as np
import ml_dtypes

import concourse.bacc as bacc
import concourse.bass as bass
import concourse.mybir as mybir
import concourse.tile as tile
from concourse.bass_utils import run_bass_kernel_spmd

B, T, E = 2048, 200, 64
H1, H2 = 80, 40
NCORES = 8
NSLOT = B // NCORES          # 256 slots per core
SGS = 16                     # slots per supergroup
NSG = NSLOT // SGS           # 16 supergroups

bf16 = mybir.dt.bfloat16
f32 = mybir.dt.float32
nbf = ml_dtypes.bfloat16
SIG = mybir.ActivationFunctionType.Sigmoid

_cache = {}


def _to_bf16(x: np.ndarray) -> np.ndarray:
    """Fast float32 -> bfloat16 (round-to-nearest-even) via integer math."""
    x = np.ascontiguousarray(x, dtype=np.float32)
    u = x.view(np.uint32)
    r = ((u + 0x7FFF + ((u >> 16) & 1)) >> 16).astype(np.uint16)
    return r.view(nbf).reshape(x.shape)


def _build(cprof):
    """cprof: tuple of NSG supergroup column counts (100 or 200)."""
    nc = bacc.Bacc(None, target_bir_lowering=False)

    xc = sum(SGS * c for c in cprof)
    kc = sum(SGS * (c // 100) * 64 for c in cprof)

    x_d = nc.dram_tensor("xf", [65, xc], bf16, kind="ExternalInput")
    k_d = nc.dram_tensor("kt", [100, kc], bf16, kind="ExternalInput")
    w_d = nc.dram_tensor("wf", [65, NSLOT * H1], bf16, kind="ExternalInput")
    w2_d = nc.dram_tensor("w2", [128, 64], bf16, kind="ExternalInput")
    w3_d = nc.dram_tensor("w3b", [128, 32], bf16, kind="ExternalInput")
    b2_d = nc.dram_tensor("b2r", [128, 1], f32, kind="ExternalInput")
    id_d = nc.dram_tensor("idn", [98, 98], bf16, kind="ExternalInput")
    out_d = nc.dram_tensor("out", [NSLOT // 32, 4, 512], f32, kind="ExternalOutput")

    with tile.TileContext(nc) as tc:
        with (
            tc.tile_pool(name="const", bufs=1) as const,
            tc.tile_pool(name="xin", bufs=4) as xin,
            tc.tile_pool(name="kin", bufs=4) as kin,
            tc.tile_pool(name="win", bufs=4) as win,
            tc.tile_pool(name="h1p", bufs=2) as h1p,
            tc.tile_pool(name="h2p", bufs=2) as h2p,
            tc.tile_pool(name="ssp", bufs=2) as ssp,
            tc.tile_pool(name="stp", bufs=6) as stp,
            tc.tile_pool(name="stg", bufs=2) as stg,
            tc.tile_pool(name="z1p", bufs=2, space=bass.MemorySpace.PSUM) as z1p,
            tc.tile_pool(name="z2p", bufs=1, space=bass.MemorySpace.PSUM) as z2p,
            tc.tile_pool(name="spp", bufs=1, space=bass.MemorySpace.PSUM) as spp,
            tc.tile_pool(name="plp", bufs=2, space=bass.MemorySpace.PSUM) as plp,
        ):
            w2_s = const.tile([128, 64], bf16)
            w3_s = const.tile([128, 32], bf16)
            b2_s = const.tile([128, 1], f32)
            id_s = const.tile([98, 98], bf16)
            nc.gpsimd.dma_start(w2_s[:], w2_d[:])
            nc.gpsimd.dma_start(w3_s[:], w3_d[:])
            nc.gpsimd.dma_start(b2_s[:], b2_d[:])
            nc.gpsimd.dma_start(id_s[:], id_d[:])

            XW = 3200   # max SGS*C columns
            for _ in range(4):
                t0 = xin.tile([128, XW], bf16, tag="xt")
                nc.vector.memset(t0[64:128, :], 0.0)
                t1 = win.tile([128, SGS * H1], bf16, tag="wt")
                nc.vector.memset(t1[64:128, :], 0.0)
            for _ in range(2):
                t2 = h1p.tile([128, XW], bf16, tag="h1t")
                nc.vector.memset(t2[64:128, :], 0.0)

            xoff = 0
            koff = 0
            poolt = None
            for g in range(NSG):
                C = cprof[g]
                nblk = C // 100
                nch = 400 // C            # slots per 400-col mm2 chunk
                chunks = SGS // nch       # 4 (C=100) or 8 (C=200)
                zb = chunks // 2          # z2 banks / mm3 count: 2 or 4
                srows = 32 * (zb - 1) + 2

                if g % 2 == 0:
                    poolt = plp.tile([97, 512], f32, tag="poolt")

                xt = xin.tile([128, XW], bf16, tag="xt")
                kt = kin.tile([100, SGS * nblk * 64], bf16, tag="ktt")
                wt = win.tile([128, SGS * H1], bf16, tag="wt")
                if g == 0:
                    nc.sync.dma_start(
                        wt[0:65, :], w_d[:, g * SGS * H1 : (g + 1) * SGS * H1]
                    )
                    q4 = SGS * C // 4
                    for qq in range(4):
                        nc.sync.dma_start(
                            xt[0:65, qq * q4 : (qq + 1) * q4],
                            x_d[:, xoff + qq * q4 : xoff + (qq + 1) * q4],
                        )
                else:
                    nc.sync.dma_start(
                        wt[0:65, :], w_d[:, g * SGS * H1 : (g + 1) * SGS * H1]
                    )
                    nc.sync.dma_start(
                        xt[0:65, 0 : SGS * C], x_d[:, xoff : xoff + SGS * C]
                    )
                nc.gpsimd.dma_start(kt[:], k_d[:, koff : koff + SGS * nblk * 64])

                # ---- layer 1: per-slot folded matmul + batched sigmoid ----
                h1t = h1p.tile([128, XW], bf16, tag="h1t")
                spb = 8 // nblk           # slots per 2-bank z1 tile
                cstr = 128 if C == 100 else 256
                for zt in range(SGS // spb):
                    z1 = z1p.tile([H1, 2, 512], f32, tag="z1")
                    for i in range(spb):
                        s = zt * spb + i
                        bb = i // (spb // 2)
                        j = i % (spb // 2)
                        nc.tensor.matmul(
                            z1[:, bb, j * cstr : j * cstr + C],
                            wt[:, s * H1 : (s + 1) * H1],
                            xt[:, s * C : (s + 1) * C],
                            start=True,
                            stop=True,
                        )
                    zv = z1[:].rearrange("p b (j c) -> p b j c", c=cstr)
                    hv = h1t[0:H1, zt * spb * C : (zt + 1) * spb * C].rearrange(
                        "p (b j c) -> p b j c", b=2, c=C
                    )
                    nc.scalar.activation(hv, zv[:, :, :, 0:C], SIG)

                # ---- layer 2 + scores ----
                h2t = h2p.tile([128, zb * 400], bf16, tag="h2t")
                spt = spp.tile([128, 512], f32, tag="spt")
                for v in range(zb):
                    z2 = z2p.tile([128, 400], f32, tag="z2")
                    nc.tensor.matmul(
                        z2[0:64, 0:400],
                        w2_s[:],
                        h1t[:, (2 * v) * 400 : (2 * v) * 400 + 400],
                        start=True,
                        stop=True,
                    )
                    nc.tensor.matmul(
                        z2[64:128, 0:400],
                        w2_s[:],
                        h1t[:, (2 * v + 1) * 400 : (2 * v + 1) * 400 + 400],
                        start=True,
                        stop=True,
                        tile_position=(0, 64),
                    )
                    nc.scalar.activation(
                        h2t[:, v * 400 : (v + 1) * 400], z2[:], SIG,
                        bias=b2_s[:, 0:1],
                    )
                    nc.tensor.matmul(
                        spt[32 * v : 32 * v + 32, 0:400],
                        w3_s[:],
                        h2t[:, v * 400 : (v + 1) * 400],
                        start=True,
                        stop=True,
                        tile_position=(0, 32 * v),
                    )

                # ---- transpose scores to token-major + pooling ----
                sst = ssp.tile([98, 400], bf16, tag="sst")
                nc.vector.tensor_copy(sst[0:srows, :], spt[0:srows, 0:400])
                stts = []
                for w in range(4):
                    stt = stp.tile([100, 98], bf16, tag="stt")
                    stts.append(stt)
                    bfv = spt[0:100, 400:449].bitcast(bf16)
                    nc.tensor.transpose(
                        bfv[:, 0:srows],
                        sst[0:srows, 100 * w : 100 * w + 100],
                        id_s[0:srows, 0:srows],
                    )
                    nc.vector.tensor_copy(stt[:, 0:srows], bfv[:, 0:srows])
                    # pooling: both blocks of a slot must be issued
                    # back-to-back (an interleaved start=True to the same
                    # psum rows clears the open group's has_written).
                    if C == 100:
                        for u in range(chunks):
                            m = 4 * u + w
                            M32 = (g % 2) * 16 + m
                            r, cc = M32 % 4, M32 // 4
                            col = 32 * (u // 2) + (u % 2)
                            nc.tensor.matmul(
                                poolt[32 * r : 32 * r + 1, 64 * cc : 64 * cc + 64],
                                stt[:, col : col + 1],
                                kt[:, (m * nblk) * 64 : (m * nblk + 1) * 64],
                                start=True,
                                stop=True,
                                tile_position=(0, 32 * r),
                            )
                    elif w % 2 == 1:
                        for u in range(chunks):
                            m = 2 * u + w // 2
                            M32 = (g % 2) * 16 + m
                            r, cc = M32 % 4, M32 // 4
                            col = 32 * (u // 2) + (u % 2)
                            for blk in (0, 1):
                                nc.tensor.matmul(
                                    poolt[32 * r : 32 * r + 1, 64 * cc : 64 * cc + 64],
                                    stts[w - 1 + blk][:, col : col + 1],
                                    kt[:, (m * nblk + blk) * 64 : (m * nblk + blk + 1) * 64],
                                    start=(blk == 0),
                                    stop=(blk == 1),
                                    tile_position=(0, 32 * r),
                                )

                xoff += SGS * C
                koff += SGS * nblk * 64

                # ---- drain pool psum every 2 supergroups ----
                if g % 2 == 1:
                    stage = stg.tile([97, 512], f32, tag="stage")
                    nc.vector.tensor_copy(stage[:], poolt[0:97, :])
                    nc.sync.dma_start(out_d[g // 2, :, :], stage[0:97:32, :])

    nc.compile()
    return nc


def kernel(query, keys, keys_length, W1, b1, W2, b2, W3, b3):
    query = np.asarray(query, np.float32)
    keys = np.asarray(keys, np.float32)
    keys_length = np.asarray(keys_length, np.int32)
    W1 = np.asarray(W1, np.float32)
    b1 = np.asarray(b1, np.float32)
    W2 = np.asarray(W2, np.float32)
    b2 = np.asarray(b2, np.float32)
    W3 = np.asarray(W3, np.float32)
    b3 = np.asarray(b3, np.float32)

    lens = keys_length[:, 0]
    order = np.argsort(lens, kind="stable")[::-1]    # descending length
    slot_max = lens[order[0::8]]                     # max len per slot
    C_s = np.where(slot_max <= 100, 100, 200)
    n1 = int((C_s == 100).sum())                     # trailing short slots
    n1f = (n1 // SGS) * SGS
    cprof = tuple(100 if g * SGS >= NSLOT - n1f else 200 for g in range(NSG))

    if cprof not in _cache:
        _cache[cprof] = _build(cprof)
    nc = _cache[cprof]

    # host-side data prep
    A = W1[0:E] + W1[2 * E : 3 * E]
    Bw = W1[E : 2 * E] - W1[2 * E : 3 * E]
    Cm = W1[3 * E : 4 * E]
    q2 = query[:, 0, :]                              # [B, E]
    mask = (np.arange(T)[None, :] < lens[:, None]).astype(np.float32)
    kmask = keys * mask[:, :, None]                  # [B, T, E]
    U = q2 @ A + b1                                  # [B, H1]
    Wf = Bw[None] + q2[:, :, None] * Cm[None]        # [B, E, H1]

    w2pad = np.zeros((128, 64), np.float32)
    w2pad[0:H1, 0:H2] = W2
    w2b = _to_bf16(w2pad)
    w3blk = np.zeros((128, 32), np.float32)
    w3blk[0:H2, 0] = W3[:, 0]
    w3blk[64 : 64 + H2, 1] = W3[:, 0]
    w3b = _to_bf16(w3blk)
    b2r = np.zeros((128, 1), np.float32)
    b2r[0:H2, 0] = b2
    b2r[64 : 64 + H2, 0] = b2
    idn = _to_bf16(np.eye(98, dtype=np.float32))

    in_maps = []
    core_ids_all = []
    for c in range(NCORES):
        ids = order[np.arange(NSLOT) * 8 + c]
        core_ids_all.append(ids)
        km = kmask[ids]                              # [256, T, E]
        wf = Wf[ids]                                 # [256, E, H1]
        uu = U[ids]                                  # [256, H1]

        xparts = []
        kparts = []
        for g in range(NSG):
            C = int(cprof[g])
            nblk = C // 100
            sl = slice(g * SGS, (g + 1) * SGS)
            kg = km[sl, 0:C, :]                      # [16, C, 64]
            xg = np.empty((65, SGS * C), np.float32)
            xg[0:64] = kg.transpose(2, 0, 1).reshape(64, SGS * C)
            xg[64] = 1.0
            xparts.append(xg)
            kparts.append(
                kg.reshape(SGS, nblk, 100, 64)
                .transpose(2, 0, 1, 3)
                .reshape(100, SGS * nblk * 64)
            )
        xf = _to_bf16(np.concatenate(xparts, axis=1))
        ktk = _to_bf16(np.concatenate(kparts, axis=1))

        wfull = np.empty((65, NSLOT * H1), np.float32)
        wfull[0:64] = wf.transpose(1, 0, 2).reshape(64, NSLOT * H1)
        wfull[64] = uu.reshape(NSLOT * H1)
        wfb = _to_bf16(wfull)

        in_maps.append({
            "xf": xf, "kt": ktk, "wf": wfb, "w2": w2b, "w3b": w3b,
            "b2r": b2r, "idn": idn,
        })

    res = run_bass_kernel_spmd(nc, in_maps, list(range(NCORES)))

    out_full = np.empty((B, E), np.float32)
    for c in range(NCORES):
        o = np.asarray(res.results[c]["out"])        # [8, 4, 512]
        rows = o.reshape(8, 4, 8, 64).transpose(0, 2, 1, 3).reshape(NSLOT, 64)
        out_full[core_ids_all[c]] = rows
    b3f = float(b3.reshape(-1)[0])
    if b3f != 0.0:
        out_full += b3f * kmask.sum(axis=1)
    return out_full.reshape(B, 1, E).astype(np.float32)
